# revision 17
# baseline (speedup 1.0000x reference)
"""Multi-head attention block (dense transformer) on 8 Trainium2 NeuronCores.

Problem: x [4, 2048, 1024] f32, w_qkv [1024, 3072], w_out [1024, 1024].
  qkv = x @ w_qkv -> split (3, 16 heads, 64) -> softmax(q k^T / 8) v -> @ w_out

Sharding: batch x head-group. Core c owns batch c//2 and heads
(c%2)*8 .. (c%2)*8+8 (4 head-pairs):
  - xT for ONE batch [1024, 2048] bf16 (4MB in vs 16MB for pure head-TP)
  - w_qkv columns for q/k/v of those 8 heads -> [1024, 1536]
  - w_out rows for those heads            -> [512, 1024]
  - each core computes a [2048, 1024] fp32 partial; host sums the 2 partials
    per batch (the all-reduce) -> 8MB out per core vs 32MB.

The ACT exp stream is the pacing resource: 256 chunk-exps of [128, 1024]
at ~1.0us each (~255us) vs ~240us of PE work, so the whole schedule is
built to keep ACT gap-free:
  - ACT runs ONLY exps; every PSUM evacuation is on DVE.
  - P2 per (hp, ni) emits 8 chunk-PAIR slots. Each slot: 4 row-group-
    packed score matmuls (both heads, K=64, bf16) + 2 exps. The av
    matmuls (bf16, lhsT=[v|1] M=65) are queued as units and popped with a
    uniform 2-slot lag THAT CROSSES ni/hp boundaries, and the accumulator
    evacuation (DVE) + reciprocal ride the same queue, so the in-order PE
    never sits on an exp wait at a boundary with ACT starving behind it.
  - P1 (projection, interleaved for hp+1) and P3 (output projection) are
    diced into ~0.9us filler units popped one-per-slot between pairs, so
    no contiguous PE block exceeds the exp slack.
P1: project qT,kT [128=2*64 rows, n] bf16 (scoresT layout) from resident
    xT tiles; vT -> v natural via the DMA crossbar transpose, one strided
    DVE copy splits the heads around ones columns (softmax sums).
P3: per token chunk, 4-deep PSUM accumulation chain over head-pairs,
    fp16 partials streamed to DRAM in 256KB stores.
Startup DMAs are released one-behind-another via write-after-write pokes
(the DMA engines round-robin among enqueued transfers, so an upfront
burst makes the first-needed tile finish last).
PSUM: 2 work + 2x2 score + 2 av = 8 banks.
"""

from collections import deque

import numpy as np
import ml_dtypes

import concourse.bacc as bacc
import concourse.tile as tile
from concourse import mybir, masks
from concourse.bass_utils import run_bass_kernel_spmd

F32 = mybir.dt.float32
BF16 = mybir.dt.bfloat16
EXP = mybir.ActivationFunctionType.Exp
F16 = mybir.dt.float16

B = 4
N = 2048             # tokens per core (one batch)
D = 1024
HEADS = 16
DH = 64
HPG = 4              # head-pairs per core (8 heads)
FT = D // 128        # 8 feature chunks
TT = 4               # token tiles (512) per batch
NI = 4               # n_i tiles of 512
NJ = 16              # n_j chunks of 128
VW = 144             # v chunk: [v_A(64) | 1 | pad7 | v_B(64) | 1 | pad]

_CACHE = {}


def build():
    nc = bacc.Bacc("TRN2", target_bir_lowering=False, debug=False, num_devices=1)
    xT_d = nc.dram_tensor("xT", [D, N], BF16, kind="ExternalInput").ap()
    wqkv_d = nc.dram_tensor("wqkv", [D, 1536], BF16, kind="ExternalInput").ap()
    wout_d = nc.dram_tensor("wout", [512, D], BF16, kind="ExternalInput").ap()
    out_d = nc.dram_tensor("out", [N, D], F16, kind="ExternalOutput").ap()
    xT_v = xT_d.rearrange("(f p) n -> f p n", p=128)

    with tile.TileContext(nc) as tc:
        with tc.tile_pool(name="const", bufs=1) as cpool, \
             tc.tile_pool(name="xt", bufs=4) as xt_pool, \
             tc.tile_pool(name="qkv", bufs=2) as qkv_pool, \
             tc.tile_pool(name="vt", bufs=2) as vt_pool, \
             tc.tile_pool(name="attn", bufs=8) as attn_pool, \
             tc.tile_pool(name="ostk", bufs=4) as ostk_pool, \
             tc.tile_pool(name="ov", bufs=4) as ov_pool, \
             tc.tile_pool(name="smol", bufs=2) as smol_pool, \
             tc.tile_pool(name="fout", bufs=2) as fout_pool, \
             tc.tile_pool(name="ps_work", bufs=2, space="PSUM") as ps_work, \
             tc.tile_pool(name="ps_score", bufs=2, space="PSUM") as ps_score, \
             tc.tile_pool(name="ps_av", bufs=2, space="PSUM") as ps_av:

            # startup DMAs: hp0's w columns + xt0 first (on separate engine
            # queues so they overlap), then the rest, wout (needed only in
            # P3) last; later loads are released behind compute via pokes.
            wv = wqkv_d.rearrange("(f p) m -> p f m", p=128)
            w_sb = cpool.tile([128, FT, 1536], BF16, tag="w")
            xt_t = {}
            for tt in range(TT):
                xt_t[tt] = xt_pool.tile([128, FT, 512], BF16, tag="xt",
                                        name=f"xt{tt}")
            xt_src = [xT_v[:, :, slice(t * 512, (t + 1) * 512)].rearrange(
                "f p n -> p f n") for t in range(TT)]
            nc.sync.dma_start(w_sb[:, 0:4, 0:384], wv[:, 0:4, 0:384])
            nc.scalar.dma_start(xt_t[0][:, 0:2, :], xt_src[0][:, 0:2, :])
            nc.gpsimd.dma_start(xt_t[0][:, 2:4, :], xt_src[0][:, 2:4, :])
            nc.scalar.copy(w_sb[0:1, 4:5, 0:1], w_sb[0:1, 0:1, 0:1])
            nc.sync.dma_start(w_sb[:, 4:8, 0:384], wv[:, 4:8, 0:384])
            nc.scalar.copy(xt_t[0][0:1, 4:5, 0:1], xt_t[0][0:1, 0:1, 0:1])
            nc.scalar.copy(xt_t[0][0:1, 5:6, 0:1], xt_t[0][0:1, 3:4, 0:1])
            nc.gpsimd.dma_start(xt_t[0][:, 4:8, :], xt_src[0][:, 4:8, :])
            wout_sb = cpool.tile([128, HPG, D], BF16, tag="wout")
            ones1 = cpool.tile([1, DH], BF16, tag="ones1")
            nc.vector.memset(ones1[:], 1.0)
            ident = cpool.tile([128, 128], BF16, tag="ident")
            masks.make_identity(nc, ident[:])

            # per-head-pair live tiles
            qT_t, kT_t, v_t, ostk_t, norm_t, pav_t = {}, {}, {}, {}, {}, {}

            # ---- filler machinery -------------------------------------
            # fill units: (est_ns, fn, deadline_slot). Deadlines are
            # global pair-slot indices (hp*32 + ni*8 + m) and pushes are
            # deadline-monotone, so a FIFO prefix pop is exact.
            fill_q = deque()

            def pop_fill(budget_ns=1150):
                while fill_q and fill_q[0][0] <= budget_ns:
                    cost, fn, _ = fill_q.popleft()
                    fn()
                    budget_ns -= cost

            def pop_force(slot):
                # deadline: run every unit this pair-slot still depends on
                while fill_q and fill_q[0][2] <= slot:
                    fill_q.popleft()[1]()

            av_q = deque()      # FIFO: ('av'|'evac', fn) — uniform lag

            def drain_av(maxlag):
                while sum(1 for k, _ in av_q if k == "av") > maxlag or \
                        (av_q and av_q[0][0] == "evac" and
                         sum(1 for k, _ in av_q if k == "av") >= maxlag):
                    av_q.popleft()[1]()

            # ---- P1: qkv projection units for one head-pair ------------
            def p1_push(hp):
                """Queue head-pair hp's projection as ~0.9us filler units
                in deadline order: [alloc, q0, k0] then per-tile k/v/xbar,
                with q1..q3 last (each q tile is first read at its own
                n_i block)."""
                b = hp * 32
                sts = [{} for _ in range(TT)]

                def alloc():
                    qT_t[hp] = qkv_pool.tile([128, N], BF16, tag="qT",
                                             name=f"qT{hp}")
                    kT_t[hp] = qkv_pool.tile([128, N], BF16, tag="kT",
                                             name=f"kT{hp}")
                    v_t[hp] = qkv_pool.tile([128, NJ, VW], BF16, tag="v",
                                            name=f"v{hp}")
                    nc.vector.memset(v_t[hp][:, :, DH::72], 1.0)

                def chain_half(tt, key, off, h, dest_fn):
                    def u():
                        st = sts[tt]
                        if h == 0:
                            st[key] = ps_work.tile(
                                [128, 512], F32, tag="work",
                                name=f"pp_{hp}_{tt}_{key}")
                        pp = st[key]
                        for ft in range(4 * h, 4 * h + 4):
                            nc.tensor.matmul(
                                pp[:], w_sb[:, ft, off:off + 128],
                                xt_t[tt][:, ft, :],
                                start=(ft == 0), stop=(ft == FT - 1))
                        if h == 1:
                            nc.vector.tensor_copy(dest_fn(), pp[:])
                    return u

                def q_units(tt, dl):
                    ts_ = slice(tt * 512, (tt + 1) * 512)
                    return [
                        (870, chain_half(tt, "q", hp * 384, 0, None), dl),
                        (1100, chain_half(tt, "q", hp * 384, 1,
                                          lambda: qT_t[hp][:, ts_]), dl)]

                def k_units(tt, dl):
                    ts_ = slice(tt * 512, (tt + 1) * 512)
                    return [
                        (870, chain_half(tt, "k", hp * 384 + 128, 0,
                                         None), dl),
                        (1100, chain_half(tt, "k", hp * 384 + 128, 1,
                                          lambda: kT_t[hp][:, ts_]), dl)]

                def v_units(tt, dl):
                    def vts_dest():
                        sts[tt]["vts"] = vt_pool.tile(
                            [128, 512], BF16, tag="vt",
                            name=f"vts_{hp}_{tt}")
                        return sts[tt]["vts"][:]

                    def xbar():
                        vnat = vt_pool.tile([128, 4, 128], BF16,
                                            tag="vnat",
                                            name=f"vnat_{hp}_{tt}")
                        nc.sync.dma_start_transpose(vnat[:],
                                                    sts[tt]["vts"][:])
                        dst = v_t[hp][:, tt * 4:(tt + 1) * 4, :].rearrange(
                            "p c (two w) -> p c two w", two=2)[:, :, :, 0:DH]
                        src = vnat[:].rearrange("p c (two w) -> p c two w",
                                                two=2)
                        nc.vector.tensor_copy(dst, src)
                    return [
                        (870, chain_half(tt, "v", hp * 384 + 256, 0,
                                         None), dl),
                        (1100, chain_half(tt, "v", hp * 384 + 256, 1,
                                          vts_dest), dl),
                        (80, xbar, dl)]

                fill_q.append((60, alloc, b))
                fill_q.extend(q_units(0, b))
                fill_q.extend(k_units(0, b))
                fill_q.extend(k_units(1, b + 2))
                fill_q.extend(v_units(0, b + 2))
                fill_q.extend(k_units(2, b + 4))
                fill_q.extend(v_units(1, b + 4))
                fill_q.extend(k_units(3, b + 6))
                fill_q.extend(v_units(2, b + 6))
                fill_q.extend(q_units(1, b + 8))
                fill_q.extend(v_units(3, b + 8))
                fill_q.extend(q_units(2, b + 16))
                fill_q.extend(q_units(3, b + 24))

            # ---- P2 ----------------------------------------------------
            def pair(hp, ni, m):
                """One chunk-pair slot: 4 packed score matmuls + 2 exps;
                queue the av unit (popped 2 slots later)."""
                qT, kT = qT_t[hp], kT_t[hp]
                qcol = slice(ni * 512, (ni + 1) * 512)
                ats = []
                for nj in (2 * m, 2 * m + 1):
                    ps = ps_score.tile([128, 1024], F32, tag="score")
                    kcol = slice(nj * 128, (nj + 1) * 128)
                    nc.tensor.matmul(ps[:, 0:512], kT[0:DH, kcol],
                                     qT[0:DH, qcol], start=True, stop=True)
                    nc.tensor.matmul(ps[:, 512:1024], kT[DH:128, kcol],
                                     qT[DH:128, qcol], start=True, stop=True)
                    at = attn_pool.tile([128, 1024], BF16, tag="attn")
                    nc.scalar.activation(at[:], ps[:], EXP, scale=0.125)
                    ats.append((nj, at))

                def avu():
                    if m == 0:
                        pav_t[(hp, ni)] = (
                            ps_av.tile([128, 512], F32, tag="av",
                                       name=f"pavA{hp}_{ni}"),
                            ps_av.tile([128, 512], F32, tag="av",
                                       name=f"pavB{hp}_{ni}"))
                    pavA, pavB = pav_t[(hp, ni)]
                    v_sb = v_t[hp]
                    for nj, at in ats:
                        nc.tensor.matmul(
                            pavA[0:DH + 1, :], v_sb[:, nj, 0:DH + 1],
                            at[:, 0:512],
                            start=(nj == 0), stop=(nj == NJ - 1))
                        nc.tensor.matmul(
                            pavB[0:DH + 1, :], v_sb[:, nj, 72:72 + DH + 1],
                            at[:, 512:1024],
                            start=(nj == 0), stop=(nj == NJ - 1))
                av_q.append(("av", avu))

            def evac_unit(hp, ni):
                def u():
                    pavA, pavB = pav_t.pop((hp, ni))
                    ovA = ov_pool.tile([DH + 1, 512], F32, tag="ov")
                    nc.vector.tensor_copy(ovA[:], pavA[0:DH + 1, :])
                    ovB = ov_pool.tile([DH + 1, 512], F32, tag="ov")
                    nc.vector.tensor_copy(ovB[:], pavB[0:DH + 1, :])
                    srow = smol_pool.tile([1, 1024], F32, tag="srow")
                    nc.vector.tensor_copy(srow[0:1, 0:512], ovA[DH:DH + 1, :])
                    nc.vector.tensor_copy(srow[0:1, 512:1024],
                                          ovB[DH:DH + 1, :])
                    rcp = smol_pool.tile([1, 1024], F32, tag="rcp")
                    nc.vector.reciprocal_approx_fast(rcp[:], srow[:])
                    norm_t[(hp, ni)] = (ovA, ovB, rcp)
                av_q.append(("evac", u))

            def p2_tail(hp, ni, pe_bcast=False):
                """Deferred normalize (DVE mul by broadcast reciprocal).
                pe_bcast replaces the two serial ~1us gpsimd broadcasts
                with K=1 ones-column matmuls — used for the final tile,
                where the PE would otherwise idle on this chain."""
                ovA, ovB, rcp = norm_t.pop((hp, ni))
                ostk = ostk_t[hp]
                ocols = slice(ni * 512, (ni + 1) * 512)
                if pe_bcast:
                    rcpb = smol_pool.tile([1, 1024], BF16, tag="rcpb")
                    nc.vector.tensor_copy(rcpb[:], rcp[:])
                    rbA = ps_av.tile([128, 512], F32, tag="av")
                    nc.tensor.matmul(rbA[0:DH, :], ones1[0:1, :],
                                     rcpb[0:1, 0:512], start=True, stop=True)
                    rbB = ps_av.tile([128, 512], F32, tag="av")
                    nc.tensor.matmul(rbB[0:DH, :], ones1[0:1, :],
                                     rcpb[0:1, 512:1024], start=True,
                                     stop=True)
                    nc.vector.tensor_mul(ostk[0:DH, ocols], rbA[0:DH, :],
                                         ovA[0:DH, :])
                    nc.vector.tensor_mul(ostk[DH:128, ocols], rbB[0:DH, :],
                                         ovB[0:DH, :])
                    return
                rbA = smol_pool.tile([DH, 512], F32, tag="rbA")
                nc.gpsimd.partition_broadcast(rbA[:], rcp[0:1, 0:512])
                rbB = smol_pool.tile([DH, 512], F32, tag="rbB")
                nc.gpsimd.partition_broadcast(rbB[:], rcp[0:1, 512:1024])
                nc.vector.tensor_mul(ostk[0:DH, ocols], rbA[:], ovA[0:DH, :])
                nc.vector.tensor_mul(ostk[DH:128, ocols], rbB[:],
                                     ovB[0:DH, :])

            # ---- P3: output projection, diced into half-chunk units ----
            def p3_units(g):
                st = {}

                def half(ch, half_i):
                    def u():
                        if ch == 0 and half_i == 0:
                            st["fo"] = fout_pool.tile([128, 2, D], F16,
                                                      tag="fout",
                                                      name=f"fo_{g}")
                        fo = st["fo"]
                        tc_ = 2 * g + ch
                        pf = ps_work.tile([128, 512], F32, tag="work")
                        for hp in range(HPG):
                            nc.tensor.matmul(
                                pf[:],
                                ostk_t[hp][:, tc_ * 128:(tc_ + 1) * 128],
                                wout_sb[:, hp,
                                        half_i * 512:(half_i + 1) * 512],
                                start=(hp == 0), stop=(hp == HPG - 1))
                        nc.vector.tensor_copy(
                            fo[:, ch, half_i * 512:(half_i + 1) * 512],
                            pf[:])
                        if ch == 1 and half_i == 1 and g < 7:
                            base = 2 * g * 128
                            nc.sync.dma_start(
                                out_d[base:base + 256, :].rearrange(
                                    "(c p) m -> p c m", p=128), fo[:])
                        elif half_i == 1 and g == 7:
                            # final group: store per chunk so the last DMA
                            # only waits on the last chunk's copies
                            base = (2 * g + ch) * 128
                            nc.sync.dma_start(
                                out_d[base:base + 128, :].rearrange(
                                    "(c p) m -> p c m", p=128),
                                fo[:, ch:ch + 1, :])
                    return u
                fill_q.extend((1000, half(ch, h), 10 ** 9)
                              for ch in range(2) for h in range(2))

            # ---- prologue: (0,0) with p1(0,*) inlined between blocks ---
            for t_ in (1, 2, 3):
                # gate on ft=7 so xt1 waits for xt0's b-half, not just a
                nc.scalar.copy(xt_t[t_][0:1, 0:1, 0:1],
                               xt_t[t_ - 1][0:1, 7:8, 0:1])
                nc.scalar.dma_start(xt_t[t_][:], xt_src[t_])
            # hp1's weight slice is small and needed by ~p1(1,0); it rides
            # right behind the x tiles
            nc.scalar.copy(w_sb[0:1, 0:1, 384:385], xt_t[3][0:1, 0:1, 0:1])
            nc.scalar.dma_start(w_sb[:, :, 384:768], wv[:, :, 384:768])

            def p1_first():
                """p1(0,0) with the three 8-ft chains interleaved ft-major:
                part A (ft 0:4) runs on the a-half DMAs alone while the
                b-halves stream in."""
                qT_t[0] = qkv_pool.tile([128, N], BF16, tag="qT", name="qT0")
                kT_t[0] = qkv_pool.tile([128, N], BF16, tag="kT", name="kT0")
                v_t[0] = qkv_pool.tile([128, NJ, VW], BF16, tag="v",
                                       name="v0")
                nc.vector.memset(v_t[0][:, :, DH::72], 1.0)
                xt = [xt_t[0][:, ft, :] for ft in range(FT)]
                vts = vt_pool.tile([128, 512], BF16, tag="vt")
                pp_v = ps_work.tile([128, 512], F32, tag="work", name="ppv")
                pp_q = ps_work.tile([128, 512], F32, tag="work", name="ppq")
                pp_k = ps_score.tile([128, 1024], F32, tag="score",
                                     name="ppk")
                trip = ((pp_v[:], 256), (pp_q[:], 0), (pp_k[:, 0:512], 128))
                for ft in range(FT):
                    for pp, off in trip:
                        nc.tensor.matmul(
                            pp, w_sb[:, ft, off:off + 128], xt[ft],
                            start=(ft == 0), stop=(ft == FT - 1))
                nc.vector.tensor_copy(qT_t[0][:, 0:512], pp_q[:])
                nc.vector.tensor_copy(kT_t[0][:, 0:512], pp_k[:, 0:512])
                nc.vector.tensor_copy(vts[:], pp_v[:])
                vnat = vt_pool.tile([128, 4, 128], BF16, tag="vnat")
                nc.sync.dma_start_transpose(vnat[:], vts[:])
                dst = v_t[0][:, 0:4, :].rearrange(
                    "p c (two w) -> p c two w", two=2)[:, :, :, 0:DH]
                src = vnat[:].rearrange("p c (two w) -> p c two w", two=2)
                nc.vector.tensor_copy(dst, src)

            def hp0_chain(off, dest_fn, tt, nm):
                pp = ps_work.tile([128, 512], F32, tag="work", name=nm)
                for ft in range(FT):
                    nc.tensor.matmul(
                        pp[:], w_sb[:, ft, off:off + 128],
                        xt_t[tt][:, ft, :],
                        start=(ft == 0), stop=(ft == FT - 1))
                nc.vector.tensor_copy(dest_fn(), pp[:])

            def hp0_q_fill(tt):
                st = {}

                def h(h_i):
                    def u():
                        if h_i == 0:
                            st["pp"] = ps_work.tile(
                                [128, 512], F32, tag="work",
                                name=f"ppq0_{tt}")
                        pp = st["pp"]
                        for ft in range(4 * h_i, 4 * h_i + 4):
                            nc.tensor.matmul(
                                pp[:], w_sb[:, ft, 0:128],
                                xt_t[tt][:, ft, :],
                                start=(ft == 0), stop=(ft == FT - 1))
                        if h_i == 1:
                            nc.vector.tensor_copy(
                                qT_t[0][:, tt * 512:(tt + 1) * 512], pp[:])
                    return u
                fill_q.append((870, h(0), tt * 8))
                fill_q.append((1100, h(1), tt * 8))

            p1_first()
            pair(0, 0, 0)
            drain_av(2)
            pair(0, 0, 1)
            drain_av(2)
            vts0 = {}
            for blk in (1, 2, 3):
                ts_ = slice(blk * 512, (blk + 1) * 512)
                hp0_chain(128, lambda: kT_t[0][:, ts_], blk, f"ppk0_{blk}")
                if blk == 3:
                    # w-rest/wout ride behind p1(0,3)'s projection (wout
                    # isn't needed until P3) so the prologue's v transposes
                    # and x tiles keep the DMA bandwidth
                    nc.scalar.copy(w_sb[0:1, 0:1, 768:769],
                                   kT_t[0][0:1, 1537:1538])
                    nc.scalar.dma_start(w_sb[:, :, 768:1536],
                                        wv[:, :, 768:1536])
                    nc.scalar.copy(wout_sb[0:1, 0:1, 0:1],
                                   w_sb[0:1, 0:1, 1535:1536])
                    nc.scalar.dma_start(
                        wout_sb[:],
                        wout_d.rearrange("(h p) m -> p h m", p=128))
                pair(0, 0, 2 * blk)
                drain_av(2)

                def vdest(blk=blk):
                    vts0[blk] = vt_pool.tile([128, 512], BF16, tag="vt",
                                             name=f"vts0_{blk}")
                    return vts0[blk][:]
                hp0_chain(256, vdest, blk, f"ppv0_{blk}")
                pair(0, 0, 2 * blk + 1)
                drain_av(2)
                vnat = vt_pool.tile([128, 4, 128], BF16, tag="vnat",
                                    name=f"vnat0_{blk}")
                nc.sync.dma_start_transpose(vnat[:], vts0[blk][:])
                dst = v_t[0][:, blk * 4:(blk + 1) * 4, :].rearrange(
                    "p c (two w) -> p c two w", two=2)[:, :, :, 0:DH]
                nc.vector.tensor_copy(
                    dst, vnat[:].rearrange("p c (two w) -> p c two w",
                                           two=2))
                hp0_q_fill(blk)
                pop_fill(900)
            ostk_t[0] = ostk_pool.tile([128, N], BF16, tag="ostk",
                                       name="ostk0")
            evac_unit(0, 0)

            # ---- steady state ------------------------------------------
            # p1(hp+1) is queued at (hp, 0, m=3) — right after hp-1's last
            # av/evac units drained, so the qkv pool WAR deps are emitted
            # in order. pop_force before each pair guarantees every unit
            # the pair depends on has been emitted (deadline prefix).
            prev = (0, 0)       # (hp, ni) whose p2_tail is pending
            for hp in range(HPG):
                for ni in range(NI):
                    if hp == 0 and ni == 0:
                        continue
                    if ni == 0:
                        ostk_t[hp] = ostk_pool.tile([128, N], BF16,
                                                    tag="ostk",
                                                    name=f"ostk{hp}")
                    if hp == 0 and ni == 1:
                        p1_push(1)
                    for m in range(8):
                        pop_force(hp * 32 + ni * 8 + m)
                        pair(hp, ni, m)
                        drain_av(2)
                        if ni == 0 and m == 3 and 1 <= hp < HPG - 1:
                            p1_push(hp + 1)
                        if m == 2 and prev is not None:
                            p2_tail(*prev)
                            if hp == HPG - 1 and prev[0] == HPG - 1:
                                p3_units(2 * prev[1])
                                p3_units(2 * prev[1] + 1)
                            prev = None
                        pop_fill()
                    evac_unit(hp, ni)
                    prev = (hp, ni)
            # leftover p3 fills (g <= 5) don't depend on the last tile's
            # avs — run them first so they overlap the final exps
            pop_fill(10 ** 9)
            drain_av(0)
            p2_tail(HPG - 1, NI - 1, pe_bcast=True)
            p3_units(2 * (NI - 1))
            p3_units(2 * (NI - 1) + 1)
            pop_fill(10 ** 9)

    nc.compile()
    return nc


def make_in_maps(x, w_qkv, w_out):
    in_maps = []
    for c in range(8):
        b, g = c // 2, c % 2
        xT_bf = np.ascontiguousarray(x[b].T).astype(ml_dtypes.bfloat16)
        # hp-major layout: [q|k|v] blocks of 128 cols per head-pair
        w_local = np.concatenate(
            [w_qkv[:, o * HEADS * DH + (g * 4 + hp) * 128:][:, :128]
             for hp in range(HPG) for o in range(3)], axis=1)
        in_maps.append({
            "xT": xT_bf,
            "wqkv": np.ascontiguousarray(w_local).astype(ml_dtypes.bfloat16),
            "wout": np.ascontiguousarray(w_out[g * 512:(g + 1) * 512, :]).astype(
                ml_dtypes.bfloat16),
        })
    return in_maps


def kernel(x, w_qkv, w_out):
    x = np.asarray(x, dtype=np.float32)
    w_qkv = np.asarray(w_qkv, dtype=np.float32)
    w_out = np.asarray(w_out, dtype=np.float32)
    if "nc" not in _CACHE:
        _CACHE["nc"] = build()
    nc = _CACHE["nc"]

    res = run_bass_kernel_spmd(nc, make_in_maps(x, w_qkv, w_out),
                               core_ids=list(range(8)))
    out = np.stack([res.results[2 * b]["out"] + res.results[2 * b + 1]["out"]
                    for b in range(B)])
    return out.astype(np.float32)


# revision 20
# speedup vs baseline: 1.0120x; 1.0120x over previous
"""Multi-head attention block (dense transformer) on 8 Trainium2 NeuronCores.

Problem: x [4, 2048, 1024] f32, w_qkv [1024, 3072], w_out [1024, 1024].
  qkv = x @ w_qkv -> split (3, 16 heads, 64) -> softmax(q k^T / 8) v -> @ w_out

Sharding: batch x head-group. Core c owns batch c//2 and heads
(c%2)*8 .. (c%2)*8+8 (4 head-pairs):
  - xT for ONE batch [1024, 2048] bf16 (4MB in vs 16MB for pure head-TP)
  - w_qkv columns for q/k/v of those 8 heads -> [1024, 1536]
  - w_out rows for those heads            -> [512, 1024]
  - each core computes a [2048, 1024] fp32 partial; host sums the 2 partials
    per batch (the all-reduce) -> 8MB out per core vs 32MB.

The ACT exp stream is the pacing resource: 256 chunk-exps of [128, 1024]
at ~1.0us each (~255us) vs ~240us of PE work, so the whole schedule is
built to keep ACT gap-free:
  - ACT runs ONLY exps; every PSUM evacuation is on DVE.
  - P2 per (hp, ni) emits 8 chunk-PAIR slots. Each slot: 4 row-group-
    packed score matmuls (both heads, K=64, bf16) + 2 exps. The av
    matmuls (bf16, lhsT=[v|1] M=65) are queued as units and popped with a
    uniform 2-slot lag THAT CROSSES ni/hp boundaries, and the accumulator
    evacuation (DVE) + reciprocal ride the same queue, so the in-order PE
    never sits on an exp wait at a boundary with ACT starving behind it.
  - P1 (projection, interleaved for hp+1) and P3 (output projection) are
    diced into ~0.9us filler units popped one-per-slot between pairs, so
    no contiguous PE block exceeds the exp slack.
P1: project qT,kT [128=2*64 rows, n] bf16 (scoresT layout) from resident
    xT tiles; vT -> v natural via the DMA crossbar transpose, one strided
    DVE copy splits the heads around ones columns (softmax sums).
P3: per token chunk, 4-deep PSUM accumulation chain over head-pairs,
    fp16 partials streamed to DRAM in 256KB stores.
Startup DMAs are released one-behind-another via write-after-write pokes
(the DMA engines round-robin among enqueued transfers, so an upfront
burst makes the first-needed tile finish last).
PSUM: 2 work + 2x2 score + 2 av = 8 banks.
"""

from collections import deque

import numpy as np
import ml_dtypes

import concourse.bacc as bacc
import concourse.tile as tile
from concourse import mybir, masks
from concourse.bass_utils import run_bass_kernel_spmd

F32 = mybir.dt.float32
BF16 = mybir.dt.bfloat16
EXP = mybir.ActivationFunctionType.Exp
F16 = mybir.dt.float16

B = 4
N = 2048             # tokens per core (one batch)
D = 1024
HEADS = 16
DH = 64
HPG = 4              # head-pairs per core (8 heads)
FT = D // 128        # 8 feature chunks
TT = 4               # token tiles (512) per batch
NI = 4               # n_i tiles of 512
NJ = 16              # n_j chunks of 128
VW = 144             # v chunk: [v_A(64) | 1 | pad7 | v_B(64) | 1 | pad]

_CACHE = {}


def build():
    nc = bacc.Bacc("TRN2", target_bir_lowering=False, debug=False, num_devices=1)
    xT_d = nc.dram_tensor("xT", [D, N], BF16, kind="ExternalInput").ap()
    wqkv_d = nc.dram_tensor("wqkv", [D, 1536], BF16, kind="ExternalInput").ap()
    wout_d = nc.dram_tensor("wout", [512, D], BF16, kind="ExternalInput").ap()
    out_d = nc.dram_tensor("out", [N, D], F16, kind="ExternalOutput").ap()
    xT_v = xT_d.rearrange("(f p) n -> f p n", p=128)

    with tile.TileContext(nc) as tc:
        with tc.tile_pool(name="const", bufs=1) as cpool, \
             tc.tile_pool(name="xt", bufs=4) as xt_pool, \
             tc.tile_pool(name="qkv", bufs=2) as qkv_pool, \
             tc.tile_pool(name="vt", bufs=2) as vt_pool, \
             tc.tile_pool(name="attn", bufs=8) as attn_pool, \
             tc.tile_pool(name="ostk", bufs=4) as ostk_pool, \
             tc.tile_pool(name="ov", bufs=4) as ov_pool, \
             tc.tile_pool(name="smol", bufs=2) as smol_pool, \
             tc.tile_pool(name="fout", bufs=2) as fout_pool, \
             tc.tile_pool(name="ps_work", bufs=2, space="PSUM") as ps_work, \
             tc.tile_pool(name="ps_score", bufs=2, space="PSUM") as ps_score, \
             tc.tile_pool(name="ps_av", bufs=2, space="PSUM") as ps_av:

            # startup DMAs: hp0's w columns + xt0 first (on separate engine
            # queues so they overlap), then the rest, wout (needed only in
            # P3) last; later loads are released behind compute via pokes.
            wv = wqkv_d.rearrange("(f p) m -> p f m", p=128)
            w_sb = cpool.tile([128, FT, 1536], BF16, tag="w")
            xt_t = {}
            for tt in range(TT):
                xt_t[tt] = xt_pool.tile([128, FT, 512], BF16, tag="xt",
                                        name=f"xt{tt}")
            xt_src = [xT_v[:, :, slice(t * 512, (t + 1) * 512)].rearrange(
                "f p n -> p f n") for t in range(TT)]
            # first-needed loads: xt0 split over scalar+gpsimd, w hp0
            # slice on sync split by q|k|v column block (the q chain can
            # start after just 0.4MB). Later loads ride the same queues
            # behind pokes so the firsts get full bandwidth.
            nc.scalar.dma_start(xt_t[0][:, 0:4, :], xt_src[0][:, 0:4, :])
            nc.gpsimd.dma_start(xt_t[0][:, 4:8, :], xt_src[0][:, 4:8, :])
            nc.sync.dma_start(w_sb[:, :, 0:128], wv[:, :, 0:128])
            nc.sync.dma_start(w_sb[:, :, 128:256], wv[:, :, 128:256])
            nc.sync.dma_start(w_sb[:, :, 256:384], wv[:, :, 256:384])
            wout_sb = cpool.tile([128, HPG, D], BF16, tag="wout")
            ones1 = cpool.tile([1, DH], BF16, tag="ones1")
            nc.vector.memset(ones1[:], 1.0)
            ident = cpool.tile([128, 128], BF16, tag="ident")
            masks.make_identity(nc, ident[:])
            # HAM warm-up: keep the PE busy while the first DMAs stream so
            # p1_first's chains run at 2.4GHz, not the cold 1.2 (the HAM
            # gate needs ~3.4us of sustained activity to open).
            garb = cpool.tile([128, 512], BF16, tag="garb")
            nc.vector.memset(garb[:], 0.0)
            warm_ps = ps_av.tile([128, 512], F32, tag="av", name="warm")
            for _ in range(16):
                nc.tensor.matmul(warm_ps[:], ident[:], garb[:],
                                 start=True, stop=True)

            # per-head-pair live tiles
            qT_t, kT_t, v_t, ostk_t, norm_t, pav_t = {}, {}, {}, {}, {}, {}

            # ---- filler machinery -------------------------------------
            # fill units: (est_ns, fn, deadline_slot). Deadlines are
            # global pair-slot indices (hp*32 + ni*8 + m) and pushes are
            # deadline-monotone, so a FIFO prefix pop is exact.
            fill_q = deque()

            def pop_fill(budget_ns=1150):
                while fill_q and fill_q[0][0] <= budget_ns:
                    cost, fn, _ = fill_q.popleft()
                    fn()
                    budget_ns -= cost

            def pop_force(slot):
                # deadline: run every unit this pair-slot still depends on
                while fill_q and fill_q[0][2] <= slot:
                    fill_q.popleft()[1]()

            av_q = deque()      # FIFO: ('av'|'evac', fn) — uniform lag

            def drain_av(maxlag):
                while sum(1 for k, _ in av_q if k == "av") > maxlag or \
                        (av_q and av_q[0][0] == "evac" and
                         sum(1 for k, _ in av_q if k == "av") >= maxlag):
                    av_q.popleft()[1]()

            # ---- P1: qkv projection units for one head-pair ------------
            def p1_push(hp):
                """Queue head-pair hp's projection as ~0.9us filler units
                in deadline order: [alloc, q0, k0] then per-tile k/v/xbar,
                with q1..q3 last (each q tile is first read at its own
                n_i block)."""
                b = hp * 32
                sts = [{} for _ in range(TT)]

                def alloc():
                    qT_t[hp] = qkv_pool.tile([128, N], BF16, tag="qT",
                                             name=f"qT{hp}")
                    kT_t[hp] = qkv_pool.tile([128, N], BF16, tag="kT",
                                             name=f"kT{hp}")
                    v_t[hp] = qkv_pool.tile([128, NJ, VW], BF16, tag="v",
                                            name=f"v{hp}")
                    nc.vector.memset(v_t[hp][:, :, DH::72], 1.0)

                def chain_half(tt, key, off, h, dest_fn):
                    def u():
                        st = sts[tt]
                        if h == 0:
                            st[key] = ps_work.tile(
                                [128, 512], F32, tag="work",
                                name=f"pp_{hp}_{tt}_{key}")
                        pp = st[key]
                        for ft in range(4 * h, 4 * h + 4):
                            nc.tensor.matmul(
                                pp[:], w_sb[:, ft, off:off + 128],
                                xt_t[tt][:, ft, :],
                                start=(ft == 0), stop=(ft == FT - 1))
                        if h == 1:
                            nc.vector.tensor_copy(dest_fn(), pp[:])
                    return u

                def q_units(tt, dl):
                    ts_ = slice(tt * 512, (tt + 1) * 512)
                    return [
                        (870, chain_half(tt, "q", hp * 384, 0, None), dl),
                        (1100, chain_half(tt, "q", hp * 384, 1,
                                          lambda: qT_t[hp][:, ts_]), dl)]

                def k_units(tt, dl):
                    ts_ = slice(tt * 512, (tt + 1) * 512)
                    return [
                        (870, chain_half(tt, "k", hp * 384 + 128, 0,
                                         None), dl),
                        (1100, chain_half(tt, "k", hp * 384 + 128, 1,
                                          lambda: kT_t[hp][:, ts_]), dl)]

                def v_units(tt, dl):
                    def vts_dest():
                        sts[tt]["vts"] = vt_pool.tile(
                            [128, 512], BF16, tag="vt",
                            name=f"vts_{hp}_{tt}")
                        return sts[tt]["vts"][:]

                    def xbar():
                        vnat = vt_pool.tile([128, 4, 128], BF16,
                                            tag="vnat",
                                            name=f"vnat_{hp}_{tt}")
                        nc.sync.dma_start_transpose(vnat[:],
                                                    sts[tt]["vts"][:])
                        dst = v_t[hp][:, tt * 4:(tt + 1) * 4, :].rearrange(
                            "p c (two w) -> p c two w", two=2)[:, :, :, 0:DH]
                        src = vnat[:].rearrange("p c (two w) -> p c two w",
                                                two=2)
                        nc.vector.tensor_copy(dst, src)
                    return [
                        (870, chain_half(tt, "v", hp * 384 + 256, 0,
                                         None), dl),
                        (1100, chain_half(tt, "v", hp * 384 + 256, 1,
                                          vts_dest), dl),
                        (80, xbar, dl)]

                fill_q.append((60, alloc, b))
                fill_q.extend(q_units(0, b))
                fill_q.extend(k_units(0, b))
                fill_q.extend(k_units(1, b + 2))
                fill_q.extend(v_units(0, b + 2))
                fill_q.extend(k_units(2, b + 4))
                fill_q.extend(v_units(1, b + 4))
                fill_q.extend(k_units(3, b + 6))
                fill_q.extend(v_units(2, b + 6))
                fill_q.extend(q_units(1, b + 8))
                fill_q.extend(v_units(3, b + 8))
                fill_q.extend(q_units(2, b + 16))
                fill_q.extend(q_units(3, b + 24))

            # ---- P2 ----------------------------------------------------
            def pair(hp, ni, m):
                """One chunk-pair slot: 4 packed score matmuls + 2 exps;
                queue the av unit (popped 2 slots later)."""
                qT, kT = qT_t[hp], kT_t[hp]
                qcol = slice(ni * 512, (ni + 1) * 512)
                ats = []
                for nj in (2 * m, 2 * m + 1):
                    ps = ps_score.tile([128, 1024], F32, tag="score")
                    kcol = slice(nj * 128, (nj + 1) * 128)
                    nc.tensor.matmul(ps[:, 0:512], kT[0:DH, kcol],
                                     qT[0:DH, qcol], start=True, stop=True)
                    nc.tensor.matmul(ps[:, 512:1024], kT[DH:128, kcol],
                                     qT[DH:128, qcol], start=True, stop=True)
                    at = attn_pool.tile([128, 1024], BF16, tag="attn")
                    nc.scalar.activation(at[:], ps[:], EXP, scale=0.125)
                    ats.append((nj, at))

                def avu():
                    if m == 0:
                        pav_t[(hp, ni)] = (
                            ps_av.tile([128, 512], F32, tag="av",
                                       name=f"pavA{hp}_{ni}"),
                            ps_av.tile([128, 512], F32, tag="av",
                                       name=f"pavB{hp}_{ni}"))
                    pavA, pavB = pav_t[(hp, ni)]
                    v_sb = v_t[hp]
                    for nj, at in ats:
                        nc.tensor.matmul(
                            pavA[0:DH + 1, :], v_sb[:, nj, 0:DH + 1],
                            at[:, 0:512],
                            start=(nj == 0), stop=(nj == NJ - 1))
                        nc.tensor.matmul(
                            pavB[0:DH + 1, :], v_sb[:, nj, 72:72 + DH + 1],
                            at[:, 512:1024],
                            start=(nj == 0), stop=(nj == NJ - 1))
                av_q.append(("av", avu))

            def evac_unit(hp, ni):
                def u():
                    pavA, pavB = pav_t.pop((hp, ni))
                    ovA = ov_pool.tile([DH + 1, 512], F32, tag="ov")
                    nc.vector.tensor_copy(ovA[:], pavA[0:DH + 1, :])
                    ovB = ov_pool.tile([DH + 1, 512], F32, tag="ov")
                    nc.vector.tensor_copy(ovB[:], pavB[0:DH + 1, :])
                    srow = smol_pool.tile([1, 1024], F32, tag="srow")
                    nc.vector.tensor_copy(srow[0:1, 0:512], ovA[DH:DH + 1, :])
                    nc.vector.tensor_copy(srow[0:1, 512:1024],
                                          ovB[DH:DH + 1, :])
                    rcp = smol_pool.tile([1, 1024], F32, tag="rcp")
                    nc.vector.reciprocal_approx_fast(rcp[:], srow[:])
                    norm_t[(hp, ni)] = (ovA, ovB, rcp)
                av_q.append(("evac", u))

            def p2_tail(hp, ni, pe_bcast=False):
                """Deferred normalize (DVE mul by broadcast reciprocal).
                pe_bcast replaces the two serial ~1us gpsimd broadcasts
                with K=1 ones-column matmuls — used for the final tile,
                where the PE would otherwise idle on this chain."""
                ovA, ovB, rcp = norm_t.pop((hp, ni))
                ostk = ostk_t[hp]
                ocols = slice(ni * 512, (ni + 1) * 512)
                if pe_bcast:
                    rcpb = smol_pool.tile([1, 1024], BF16, tag="rcpb")
                    nc.vector.tensor_copy(rcpb[:], rcp[:])
                    rbA = ps_av.tile([128, 512], F32, tag="av")
                    nc.tensor.matmul(rbA[0:DH, :], ones1[0:1, :],
                                     rcpb[0:1, 0:512], start=True, stop=True)
                    rbB = ps_av.tile([128, 512], F32, tag="av")
                    nc.tensor.matmul(rbB[0:DH, :], ones1[0:1, :],
                                     rcpb[0:1, 512:1024], start=True,
                                     stop=True)
                    nc.vector.tensor_mul(ostk[0:DH, ocols], rbA[0:DH, :],
                                         ovA[0:DH, :])
                    nc.vector.tensor_mul(ostk[DH:128, ocols], rbB[0:DH, :],
                                         ovB[0:DH, :])
                    return
                rbA = smol_pool.tile([DH, 512], F32, tag="rbA")
                nc.gpsimd.partition_broadcast(rbA[:], rcp[0:1, 0:512])
                rbB = smol_pool.tile([DH, 512], F32, tag="rbB")
                nc.gpsimd.partition_broadcast(rbB[:], rcp[0:1, 512:1024])
                nc.vector.tensor_mul(ostk[0:DH, ocols], rbA[:], ovA[0:DH, :])
                nc.vector.tensor_mul(ostk[DH:128, ocols], rbB[:],
                                     ovB[0:DH, :])

            # ---- P3: output projection, diced into half-chunk units ----
            def p3_units(g):
                st = {}

                def half(ch, half_i):
                    def u():
                        if ch == 0 and half_i == 0:
                            st["fo"] = fout_pool.tile([128, 2, D], F16,
                                                      tag="fout",
                                                      name=f"fo_{g}")
                        fo = st["fo"]
                        tc_ = 2 * g + ch
                        pf = ps_work.tile([128, 512], F32, tag="work")
                        for hp in range(HPG):
                            nc.tensor.matmul(
                                pf[:],
                                ostk_t[hp][:, tc_ * 128:(tc_ + 1) * 128],
                                wout_sb[:, hp,
                                        half_i * 512:(half_i + 1) * 512],
                                start=(hp == 0), stop=(hp == HPG - 1))
                        nc.vector.tensor_copy(
                            fo[:, ch, half_i * 512:(half_i + 1) * 512],
                            pf[:])
                        if ch == 1 and half_i == 1 and g < 7:
                            base = 2 * g * 128
                            nc.sync.dma_start(
                                out_d[base:base + 256, :].rearrange(
                                    "(c p) m -> p c m", p=128), fo[:])
                        elif half_i == 1 and g == 7:
                            # final group: store per chunk so the last DMA
                            # only waits on the last chunk's copies
                            base = (2 * g + ch) * 128
                            nc.sync.dma_start(
                                out_d[base:base + 128, :].rearrange(
                                    "(c p) m -> p c m", p=128),
                                fo[:, ch:ch + 1, :])
                    return u
                fill_q.extend((1000, half(ch, h), 10 ** 9)
                              for ch in range(2) for h in range(2))

            # ---- prologue: (0,0) with p1(0,*) inlined between blocks ---
            # xt1/xt2 ride behind xt0-a on scalar, xt3 + the remaining
            # weights behind xt0-b on gpsimd; each release poke writes
            # into the next transfer's destination so the tile framework
            # serializes them without touching other queues.
            nc.scalar.copy(xt_t[1][0:1, 0:1, 0:1], xt_t[0][0:1, 3:4, 0:1])
            nc.scalar.dma_start(xt_t[1][:], xt_src[1])
            nc.scalar.copy(xt_t[2][0:1, 0:1, 0:1], xt_t[1][0:1, 7:8, 0:1])
            nc.scalar.dma_start(xt_t[2][:], xt_src[2])
            nc.scalar.copy(xt_t[3][0:1, 0:1, 0:1], xt_t[0][0:1, 7:8, 0:1])
            nc.gpsimd.dma_start(xt_t[3][:], xt_src[3])
            nc.scalar.copy(w_sb[0:1, 0:1, 384:385], xt_t[3][0:1, 7:8, 0:1])
            nc.gpsimd.dma_start(w_sb[:, :, 384:768], wv[:, :, 384:768])

            def p1_first():
                """p1(0,0): q and k chains first (scores can start), then
                v. All of xt0 is resident by the time the warm-up matmuls
                drain, so the chains run straight through."""
                qT_t[0] = qkv_pool.tile([128, N], BF16, tag="qT", name="qT0")
                kT_t[0] = qkv_pool.tile([128, N], BF16, tag="kT", name="kT0")
                v_t[0] = qkv_pool.tile([128, NJ, VW], BF16, tag="v",
                                       name="v0")
                nc.vector.memset(v_t[0][:, :, DH::72], 1.0)
                vts = vt_pool.tile([128, 512], BF16, tag="vt")
                for off, dest in ((0, qT_t[0][:, 0:512]),
                                  (128, kT_t[0][:, 0:512]),
                                  (256, vts[:])):
                    pp = ps_work.tile([128, 512], F32, tag="work",
                                      name=f"ppf_{off}")
                    for ft in range(FT):
                        nc.tensor.matmul(
                            pp[:], w_sb[:, ft, off:off + 128],
                            xt_t[0][:, ft, :],
                            start=(ft == 0), stop=(ft == FT - 1))
                    nc.vector.tensor_copy(dest, pp[:])
                vnat = vt_pool.tile([128, 4, 128], BF16, tag="vnat")
                nc.sync.dma_start_transpose(vnat[:], vts[:])
                dst = v_t[0][:, 0:4, :].rearrange(
                    "p c (two w) -> p c two w", two=2)[:, :, :, 0:DH]
                src = vnat[:].rearrange("p c (two w) -> p c two w", two=2)
                nc.vector.tensor_copy(dst, src)

            def hp0_chain(off, dest_fn, tt, nm):
                pp = ps_work.tile([128, 512], F32, tag="work", name=nm)
                for ft in range(FT):
                    nc.tensor.matmul(
                        pp[:], w_sb[:, ft, off:off + 128],
                        xt_t[tt][:, ft, :],
                        start=(ft == 0), stop=(ft == FT - 1))
                nc.vector.tensor_copy(dest_fn(), pp[:])

            def hp0_q_fill(tt):
                st = {}

                def h(h_i):
                    def u():
                        if h_i == 0:
                            st["pp"] = ps_work.tile(
                                [128, 512], F32, tag="work",
                                name=f"ppq0_{tt}")
                        pp = st["pp"]
                        for ft in range(4 * h_i, 4 * h_i + 4):
                            nc.tensor.matmul(
                                pp[:], w_sb[:, ft, 0:128],
                                xt_t[tt][:, ft, :],
                                start=(ft == 0), stop=(ft == FT - 1))
                        if h_i == 1:
                            nc.vector.tensor_copy(
                                qT_t[0][:, tt * 512:(tt + 1) * 512], pp[:])
                    return u
                fill_q.append((870, h(0), tt * 8))
                fill_q.append((1100, h(1), tt * 8))

            p1_first()
            pair(0, 0, 0)
            drain_av(2)
            pair(0, 0, 1)
            drain_av(2)
            vts0 = {}
            for blk in (1, 2, 3):
                ts_ = slice(blk * 512, (blk + 1) * 512)
                hp0_chain(128, lambda: kT_t[0][:, ts_], blk, f"ppk0_{blk}")
                if blk == 3:
                    # w-rest/wout ride behind the earlier gpsimd loads
                    # (wout isn't needed until P3); the release pokes are
                    # DVE writes into the DMA destinations, keeping the
                    # Scalar queue exp-pure
                    nc.vector.tensor_copy(w_sb[0:1, 0:1, 768:769],
                                          w_sb[0:1, 0:1, 384:385])
                    nc.gpsimd.dma_start(w_sb[:, :, 768:1536],
                                        wv[:, :, 768:1536])
                    nc.vector.tensor_copy(wout_sb[0:1, 0:1, 0:1],
                                          w_sb[0:1, 0:1, 1535:1536])
                    nc.gpsimd.dma_start(
                        wout_sb[:],
                        wout_d.rearrange("(h p) m -> p h m", p=128))
                pair(0, 0, 2 * blk)
                drain_av(2)

                def vdest(blk=blk):
                    vts0[blk] = vt_pool.tile([128, 512], BF16, tag="vt",
                                             name=f"vts0_{blk}")
                    return vts0[blk][:]
                hp0_chain(256, vdest, blk, f"ppv0_{blk}")
                pair(0, 0, 2 * blk + 1)
                drain_av(2)
                vnat = vt_pool.tile([128, 4, 128], BF16, tag="vnat",
                                    name=f"vnat0_{blk}")
                nc.sync.dma_start_transpose(vnat[:], vts0[blk][:])
                dst = v_t[0][:, blk * 4:(blk + 1) * 4, :].rearrange(
                    "p c (two w) -> p c two w", two=2)[:, :, :, 0:DH]
                nc.vector.tensor_copy(
                    dst, vnat[:].rearrange("p c (two w) -> p c two w",
                                           two=2))
                hp0_q_fill(blk)
                pop_fill(900)
            ostk_t[0] = ostk_pool.tile([128, N], BF16, tag="ostk",
                                       name="ostk0")
            evac_unit(0, 0)

            # ---- steady state ------------------------------------------
            # p1(hp+1) is queued at (hp, 0, m=3) — right after hp-1's last
            # av/evac units drained, so the qkv pool WAR deps are emitted
            # in order. pop_force before each pair guarantees every unit
            # the pair depends on has been emitted (deadline prefix).
            prev = (0, 0)       # (hp, ni) whose p2_tail is pending
            for hp in range(HPG):
                for ni in range(NI):
                    if hp == 0 and ni == 0:
                        continue
                    if ni == 0:
                        ostk_t[hp] = ostk_pool.tile([128, N], BF16,
                                                    tag="ostk",
                                                    name=f"ostk{hp}")
                    if hp == 0 and ni == 1:
                        p1_push(1)
                    for m in range(8):
                        pop_force(hp * 32 + ni * 8 + m)
                        pair(hp, ni, m)
                        drain_av(2)
                        if ni == 0 and m == 3 and 1 <= hp < HPG - 1:
                            p1_push(hp + 1)
                        if m == 2 and prev is not None:
                            p2_tail(*prev)
                            if hp == HPG - 1 and prev[0] == HPG - 1:
                                p3_units(2 * prev[1])
                                p3_units(2 * prev[1] + 1)
                            prev = None
                        pop_fill()
                    evac_unit(hp, ni)
                    prev = (hp, ni)
            # leftover p3 fills (g <= 5) don't depend on the last tile's
            # avs — run them first so they overlap the final exps
            pop_fill(10 ** 9)
            drain_av(0)
            p2_tail(HPG - 1, NI - 1, pe_bcast=True)
            p3_units(2 * (NI - 1))
            p3_units(2 * (NI - 1) + 1)
            pop_fill(10 ** 9)

    nc.compile()
    return nc


def make_in_maps(x, w_qkv, w_out):
    in_maps = []
    for c in range(8):
        b, g = c // 2, c % 2
        xT_bf = np.ascontiguousarray(x[b].T).astype(ml_dtypes.bfloat16)
        # hp-major layout: [q|k|v] blocks of 128 cols per head-pair
        w_local = np.concatenate(
            [w_qkv[:, o * HEADS * DH + (g * 4 + hp) * 128:][:, :128]
             for hp in range(HPG) for o in range(3)], axis=1)
        in_maps.append({
            "xT": xT_bf,
            "wqkv": np.ascontiguousarray(w_local).astype(ml_dtypes.bfloat16),
            "wout": np.ascontiguousarray(w_out[g * 512:(g + 1) * 512, :]).astype(
                ml_dtypes.bfloat16),
        })
    return in_maps


def kernel(x, w_qkv, w_out):
    x = np.asarray(x, dtype=np.float32)
    w_qkv = np.asarray(w_qkv, dtype=np.float32)
    w_out = np.asarray(w_out, dtype=np.float32)
    if "nc" not in _CACHE:
        _CACHE["nc"] = build()
    nc = _CACHE["nc"]

    res = run_bass_kernel_spmd(nc, make_in_maps(x, w_qkv, w_out),
                               core_ids=list(range(8)))
    out = np.stack([res.results[2 * b]["out"] + res.results[2 * b + 1]["out"]
                    for b in range(B)])
    return out.astype(np.float32)
